# revision 13
# baseline (speedup 1.0000x reference)
"""DBOW embedding-lookup kernel for Trainium2 (8 NeuronCores, SPMD).

Computes scores[b, k] = dot(D[doc_ids[b]], O[:, target_noise_ids[b, k]])
for B=16384, K=26, V=128, over doc table D [1e6, 128] f32 and word table
O [128, 1e5] f32.

Strategy: data-parallel over batch (2048 rows per core, 16 tiles of 128).
Host transposes O once to OT [1e5, 128] bf16 (256B rows).

The previous revision issued one indirect_dma_start per gathered row
(432/core); each pays the Q7 SWDGE per-instruction desc-gen launch
(~1.4us), serializing to ~614us. The HW indirect1d path supports only
ONE index per partition per instruction, so that is a hard floor. The
word gathers instead use the extended InstDMAGatherAnt ucode (library
`mlp`), whose desc-gen is vectorized 16-wide across the Q7 cpu pair of
its queue (994ns + ~0.34ns/row per instruction).

dma_gather constraints shape the layout:
  - int16 indices (0..32767) -> OT is split into 4 chunks (bases
    0/32768/65536/98304); each instruction reads one chunk with
    chunk-local indices.
  - dest mapping is fixed: idx position j -> (partition j%128, column
    j//128). To keep the doc vector d aligned (partition = batch row),
    every dest column is statically dedicated to one (tile, chunk): per
    tile, chunk c owns M[c] columns (M = 12,12,12,4; 40 cols/tile vs 26
    real pairs = 1.54x gather inflation from per-partition padding).
    Rows with more than M[c] ids in chunk c (~1% of pairs for uniform
    random ids) are computed on the host during output assembly.
  - the SWDGE descriptor ring holds only ~80 descriptors per ring
    (empirically: 1024-idx instructions work, 1280 hang the core), so
    instructions are capped at 8 columns = 1024 idxs. Each chunk's
    16*M[c] per-tile columns are chopped globally into 8-col
    instructions (80 per core), round-robined over the 4 SWDGE queues
    (each queue has its own rings and Q7 cpu pair). DMA completion
    order is only FIFO per queue, so each queue gets its own counting
    semaphore.
  - index lists are wrapped [16, n/16] (idx j at partition j%16, free
    j//16) and replicated across all 128 partitions (each Q7 cpu reads
    its own 16-partition slice).

Per core: 16 single-index-per-partition indirect DMAs gather the doc
vectors (f32->bf16 in-DMA cast) into a resident [128, 2048] bf16 tile;
then 80 dma_gather instructions stream word rows into a 16-slot ring;
DVE does a bf16 broadcast multiply (2x mode) + f32 reduce per
(tile, chunk-col-range) segment; one contiguous store per repeat.
"""

import numpy as np
import ml_dtypes

import concourse.bass as bass
import concourse.mybir as mybir
from concourse.bass_utils import run_bass_kernel_spmd
from concourse.library_overlay import lower_extended_insts
from concourse import library_config


# --- compat shims for the walrus build in this container ---------------------
# 1) clear_and_free_semaphores emits EVENT_SEMAPHORE_RANGE_CLEAR + a
#    multi-wait Drain; this walrus rejects both encodings. With a single
#    context per program the freed sems are never reused, so the cleanup
#    instructions are dead weight — keep only the bookkeeping.
def _patched_clear_and_free(self, sems):
    if not sems:
        return
    sem_nums = [s.num if hasattr(s, "num") else s for s in sems]
    self._state.prepend_free_semaphores(sem_nums)
    for ps in self._tile_sem_poison_stack:
        ps.update(sem_nums)


bass.Bass.clear_and_free_semaphores = _patched_clear_and_free


# 2) This walrus encodes at most ONE sync-wait per instruction; block exits
#    can attach several. Split the extras into wait-only NoOps on the same
#    engine just before the instruction (same-engine program order preserves
#    semantics).
def _split_multi_waits(nc):
    n_new = 0
    for f in nc.m.functions:
        for bb in f.blocks:
            out = []
            changed = False
            for inst in bb.instructions:
                si = inst.sync_info
                waits = list(si.on_wait) if si is not None and si.on_wait else []
                if len(waits) > 1:
                    changed = True
                    for w in waits[:-1]:
                        nop = mybir.InstNoOp(
                            name=f"{inst.name}_w{n_new}", ins=[], outs=[]
                        )
                        n_new += 1
                        nop.engine = inst.engine
                        nop.sync_info = mybir.SyncInfo(on_wait=[w], on_update=[])
                        out.append(nop)
                    inst.sync_info = mybir.SyncInfo(
                        on_wait=[waits[-1]],
                        on_update=list(si.on_update) if si.on_update else [],
                    )
                out.append(inst)
            if changed:
                bb.instructions = out
    return n_new


VEC = 128
NUM_DOCS = 1_000_000
NUM_WORDS = 100_000
BATCH = 16_384
K = 26
NCORES = 8
BLOC = BATCH // NCORES          # 2048 batch rows per core
NTILES = BLOC // 128            # 16 tiles of 128 rows

CH = 4                          # word-table chunks (int16 index reach)
CH_BASES = (0, 32_768, 65_536, 98_304)
CH_ROWS = (32_768, 32_768, 32_768, NUM_WORDS - 98_304)
M = (12, 12, 12, 4)             # dest columns per (tile, chunk)
COFF = (0, 12, 24, 36)          # chunk column offset within a tile
COLS = sum(M)                   # 40 columns per tile
GC = 8                          # columns per gather instruction (ring cap)
NQ = 1                          # SWDGE queues (multi-queue ring setup is broken in this runtime)

# instruction plan: for chunk c, flat cols fc = t*M[c]+g chopped into GC-wide
# instructions; each instruction lists (tile, g_start, ncols, col_in_insn)
# segments it covers.
_PLAN = []  # list of (chunk, [(tile, g0, n, insn_col0), ...])
for _c in range(CH):
    total = NTILES * M[_c]
    for fc0 in range(0, total, GC):
        segs = []
        fc = fc0
        while fc < min(fc0 + GC, total):
            t, g = divmod(fc, M[_c])
            n = min(M[_c] - g, fc0 + GC - fc)
            segs.append((t, g, n, fc - fc0))
            fc += n
        _PLAN.append((_c, segs))
NW = len(_PLAN)                 # 80 gather instructions per core
NIDX = 128 * GC                 # idxs per instruction (uniform; tail padded)
IDXF = NIDX // 16               # int16 free extent per instruction
WIDX_FREE = NW * IDXF

F32 = mybir.dt.float32
BF16 = mybir.dt.bfloat16
I32 = mybir.dt.int32
I16 = mybir.dt.int16

_cached = {}


def _build_program(repeat=1, mode="full"):
    nc = bass.Bass(trn_type="TRN2")

    D_t = nc.dram_tensor("D", [NUM_DOCS, VEC], F32, kind="ExternalInput")
    OT_t = nc.dram_tensor("OT", [NUM_WORDS, VEC], BF16, kind="ExternalInput")
    did_t = nc.dram_tensor("did_T", [128, NTILES], I32, kind="ExternalInput")
    widx_t = nc.dram_tensor("widx", [128, WIDX_FREE], I16, kind="ExternalInput")
    out_t = nc.dram_tensor("out", [128, NTILES * COLS], F32, kind="ExternalOutput")

    s_ids = nc.alloc_semaphore("s_ids")
    s_d = nc.alloc_semaphore("s_d")
    s_wq = [nc.alloc_semaphore(f"s_w{q}") for q in range(NQ)]
    s_mult = nc.alloc_semaphore("s_mult")
    s_red = nc.alloc_semaphore("s_red")
    s_out = nc.alloc_semaphore("s_out")

    N = repeat * NW
    NSLOT = 16
    SLOTW = GC * VEC            # bf16 elems per ring slot

    with (
        nc.sbuf_tensor([128, NTILES], I32) as did,
        nc.sbuf_tensor([128, WIDX_FREE], I16) as widx,
        nc.sbuf_tensor([128, NTILES * VEC], BF16) as d_all,
        nc.sbuf_tensor([128, NSLOT * SLOTW], BF16) as wring,
        nc.sbuf_tensor([128, GC * VEC], BF16) as prod,
        nc.sbuf_tensor([128, NTILES * COLS], F32) as scores,
        nc.Block(),
    ):
        # --- sync (SP, HWDGE): id loads ------------------------------------
        nc.sync.dma_start(out=widx[:], in_=widx_t[:, :]).then_inc(s_ids, 16)
        nc.sync.dma_start(out=did[:], in_=did_t[:, :]).then_inc(s_ids, 16)

        # --- Pool (SWDGE): doc gathers, then chunked word gathers ----------
        nc.gpsimd.wait_ge(s_ids, 32)
        nc.gpsimd.load_library(library_config.mlp)
        nidx_reg = nc.gpsimd.to_reg(NIDX)
        for t in range(NTILES):
            nc.gpsimd.indirect_dma_start(
                out=d_all[:, t * VEC : (t + 1) * VEC],
                out_offset=None,
                in_=D_t[:],
                in_offset=bass.IndirectOffsetOnAxis(ap=did[:, t : t + 1], axis=0),
            ).then_inc(s_d, 16)
        # Drain the mainline (indirect) descriptors from the shared SWDGE
        # rings before the extended dma_gather stream starts writing its own:
        # mixing the two paths with descriptors in flight intermittently
        # corrupts gathers.
        nc.gpsimd.wait_ge(s_d, 16 * NTILES)
        for n in range(N):
            i = n % NW
            c, _segs = _PLAN[i]
            q = i % NQ
            slot = n % NSLOT
            if n >= NSLOT and mode != "gather":
                nc.gpsimd.wait_ge(s_mult, n - NSLOT + 1)
            nc.gpsimd.dma_gather(
                wring[:, slot * SLOTW : (slot + 1) * SLOTW].rearrange(
                    "p (g v) -> p g v", v=VEC
                ),
                OT_t[CH_BASES[c] : CH_BASES[c] + CH_ROWS[c]],
                widx[:, i * IDXF : (i + 1) * IDXF],
                NIDX,
                nidx_reg,
                VEC,
                queue_num=q,
            ).then_inc(s_wq[q], 16)

        # --- DVE: multiply + reduce; sync: stores --------------------------
        if mode != "gather":
            qcnt = [0] * NQ
            nc.vector.wait_ge(s_d, 16 * NTILES)
            for n in range(N):
                r, i = divmod(n, NW)
                c, segs = _PLAN[i]
                q = i % NQ
                qcnt[q] += 1
                slot = n % NSLOT
                nc.vector.wait_ge(s_wq[q], 16 * qcnt[q])
                if r > 0 and i == 0:
                    nc.vector.wait_ge(s_out, 16 * r)
                for si, (t, g0, ncol, ic0) in enumerate(segs):
                    d3 = (
                        d_all[:, t * VEC : (t + 1) * VEC]
                        .unsqueeze(1)
                        .broadcast_to([128, ncol, VEC])
                    )
                    wv = wring[
                        :,
                        slot * SLOTW + ic0 * VEC : slot * SLOTW
                        + (ic0 + ncol) * VEC,
                    ]
                    mul = nc.vector.tensor_tensor(
                        out=prod[:, : ncol * VEC].rearrange(
                            "p (m v) -> p m v", v=VEC
                        ),
                        in0=wv.rearrange("p (m v) -> p m v", v=VEC),
                        in1=d3,
                        op=mybir.AluOpType.mult,
                    )
                    if si == len(segs) - 1:
                        mul.then_inc(s_mult, 1)
                    red = nc.vector.tensor_reduce(
                        out=scores[
                            :,
                            t * COLS + COFF[c] + g0 : t * COLS + COFF[c] + g0 + ncol,
                        ],
                        in_=prod[:, : ncol * VEC].rearrange(
                            "p (m v) -> p m v", v=VEC
                        ),
                        axis=mybir.AxisListType.X,
                        op=mybir.AluOpType.add,
                    )
                    if si == len(segs) - 1:
                        red.then_inc(s_red, 1)

            for r in range(repeat):
                nc.sync.wait_ge(s_red, NW * (r + 1))
                nc.sync.dma_start(out=out_t[:, :], in_=scores[:, :]).then_inc(
                    s_out, 16
                )
            nc.sync.wait_ge(s_out, 16 * repeat)
        else:
            for q in range(NQ):
                nq_total = sum(1 for i in range(NW) if i % NQ == q)
                nc.vector.wait_ge(s_wq[q], 16 * nq_total * repeat)

    _split_multi_waits(nc)
    lower_extended_insts(nc)
    return nc


def _get_program(repeat=1, mode="full"):
    key = (repeat, mode)
    if key not in _cached:
        _cached[key] = _build_program(repeat, mode)
    return _cached[key]


_host_cache = {}


def _prep_core(noise_c):
    """Bucket one core's noise ids [NTILES, 128, K] into the chunked
    column layout. Returns (widx [128, WIDX_FREE] int16,
    colmap [NTILES, 128, K] int64 with -1 for overflow)."""
    ch = noise_c >> 15                      # chunk id 0..3
    local = (noise_c & 0x7FFF).astype(np.int64)

    order = np.argsort(ch, axis=2, kind="stable")
    s_ch = np.take_along_axis(ch, order, 2)         # sorted chunk ids
    s_local = np.take_along_axis(local, order, 2)
    s_k = order                                      # original k per slot

    # rank g within each (t, p, chunk) run of the sorted list
    cnt = np.stack([(ch == c).sum(2) for c in range(CH)], axis=2)  # [T,128,4]
    starts = np.zeros_like(cnt)
    starts[:, :, 1:] = np.cumsum(cnt, axis=2)[:, :, :-1]
    pos = np.broadcast_to(np.arange(K), s_ch.shape)
    g = pos - np.take_along_axis(starts, s_ch, 2)

    Marr = np.asarray(M)
    valid = g < Marr[s_ch]

    colmap = np.full((NTILES, 128, K), -1, np.int64)
    t_idx, p_idx, _ = np.indices(s_ch.shape)
    col = t_idx * COLS + np.asarray(COFF)[s_ch] + g
    np.put_along_axis(colmap, s_k, np.where(valid, col, -1), axis=2)

    # per-instruction index arrays: chunk c's flat col fc = t*M_c + g,
    # instruction i covers fc in [fc0, fc0+GC); position j = (fc-fc0)*128+p
    widx = np.zeros((128, WIDX_FREE), np.int16)
    fc_all = t_idx * Marr[s_ch] + g                  # flat col within chunk
    insn_base = {}
    ib = 0
    for c in range(CH):
        insn_base[c] = ib
        ib += (NTILES * M[c] + GC - 1) // GC
    for c in range(CH):
        sel = valid & (s_ch == c)
        tt, pp, ss = np.nonzero(sel)
        fc = fc_all[tt, pp, ss]
        insn = insn_base[c] + fc // GC
        j = (fc % GC) * 128 + pp
        vals = s_local[tt, pp, ss].astype(np.int16)
        flat = np.zeros(NW * NIDX, np.int16)
        flat[insn * NIDX + j] = vals
        for i in np.unique(insn):
            arr = flat[i * NIDX : (i + 1) * NIDX]
            wrapped = arr.reshape(IDXF, 16).T        # [16, n/16]
            widx[:, i * IDXF : (i + 1) * IDXF] = np.tile(wrapped, (8, 1))
    return widx, colmap


def _make_in_maps(context_ids, doc_ids, target_noise_ids, D, O):
    D = np.ascontiguousarray(np.asarray(D, dtype=np.float32))
    okey = id(O)
    if _host_cache.get("okey") != okey:
        _host_cache["okey"] = okey
        _host_cache["OT"] = np.ascontiguousarray(
            np.asarray(O, dtype=np.float32).T.astype(ml_dtypes.bfloat16)
        )
    OT = _host_cache["OT"]
    doc_ids = np.asarray(doc_ids, dtype=np.int32)
    noise = np.asarray(target_noise_ids, dtype=np.int32)

    in_maps = []
    colmaps = []
    for c in range(NCORES):
        sl = slice(c * BLOC, (c + 1) * BLOC)
        did_T = np.ascontiguousarray(doc_ids[sl].reshape(NTILES, 128).T)
        widx, colmap = _prep_core(noise[sl].reshape(NTILES, 128, K))
        in_maps.append({"D": D, "OT": OT, "did_T": did_T, "widx": widx})
        colmaps.append(colmap)
    return in_maps, colmaps


def _assemble(results, colmaps, doc_ids, noise, D, O):
    """Per-core [128, NTILES*COLS] buffers -> full [BATCH, K] scores,
    host-computing the rare overflow pairs."""
    out = np.empty((BATCH, K), np.float32)
    parts = np.arange(128)
    for c in range(NCORES):
        buf = np.asarray(results[c]["out"])         # [128, NTILES*COLS]
        colmap = colmaps[c]                          # [NTILES, 128, K]
        vals = buf[parts[None, :, None], np.maximum(colmap, 0)]
        out[c * BLOC : (c + 1) * BLOC] = vals.reshape(BLOC, K)
        ov_t, ov_p, ov_k = np.nonzero(colmap < 0)
        if len(ov_t):
            b = c * BLOC + ov_t * 128 + ov_p
            ids = noise[b, ov_k]
            d = D[doc_ids[b]].astype(np.float32)
            w = O[:, ids].T.astype(np.float32)
            out[b, ov_k] = np.einsum("nv,nv->n", d, w)
    return out


def run(inputs, trace=False, repeat=1, mode="full", **kw):
    """Run the SPMD kernel; returns (full_output, BassKernelResults)."""
    nc = _get_program(repeat, mode)
    in_maps, colmaps = _make_in_maps(**inputs)
    res = run_bass_kernel_spmd(
        nc, in_maps, core_ids=list(range(NCORES)), trace=trace, **kw
    )
    out = _assemble(
        res.results,
        colmaps,
        np.asarray(inputs["doc_ids"], dtype=np.int64),
        np.asarray(inputs["target_noise_ids"], dtype=np.int64),
        np.asarray(inputs["D"], dtype=np.float32),
        np.asarray(inputs["O"], dtype=np.float32),
    )
    return out, res


def kernel(**inputs):
    out, _ = run(inputs, trace=False)
    return out
